# revision 7
# baseline (speedup 1.0000x reference)
"""Trainium2 Bass kernel for C = tril(A @ B), A/B lower-triangular 4096x4096 fp32.

Distribution: 32 row-blocks of 128 interleaved mod 8 across cores; core c owns
blocks {c, 8+c, 16+c, 24+c} (slots t=0..3). Slot t uses a uniform K/column
bound of 8*(t+1) k-blocks so every core runs the identical SPMD program; the
inputs are exactly triangular, so the over-computed region contributes exact
zeros (no masking needed anywhere).

Per core: A^T column-band (host-transposed, [4096, 512]) stays SBUF-resident;
B tiles [128, 512] are streamed once each (only tiles with k >= 4*jt, the
nonzero triangle); PSUM accumulates over the trimmed K range per (slot,
col-tile); results DMA straight from PSUM to DRAM. Upper-triangle columns
beyond each slot's bound are never written and stay zero via the pre-zeroed
output buffers.
"""

import numpy as np

N = 4096
P = 128
NCORES = 8
SLOTS = 4          # row-blocks per core
KB = N // P        # 32 k-blocks
CW = 512           # output col-tile width (= max fp32 matmul free dim)
NJT = N // CW      # 8 col tiles

MM_DT_NAME = "float32r"  # "float32" (4 cyc/row) or "float32r" (1 cyc/row)

_cached = {}


def _build(mm_dt_name):
    import concourse.mybir as mybir
    import concourse.tile as tile
    from concourse import bacc

    mm_dt = getattr(mybir.dt, mm_dt_name)

    nc = bacc.Bacc("TRN2", target_bir_lowering=False, debug=False,
                   num_devices=NCORES)
    at_d = nc.dram_tensor("at", [N, SLOTS * P], mm_dt, kind="ExternalInput").ap()
    b_d = nc.dram_tensor("b", [N, N], mm_dt, kind="ExternalInput").ap()
    o_d = nc.dram_tensor("o", [SLOTS, P, N], mybir.dt.float32,
                         kind="ExternalOutput").ap()

    with tile.TileContext(nc) as tc:
        with (
            tc.tile_pool(name="atp", bufs=1) as atp,
            tc.tile_pool(name="bp", bufs=4) as bp,
            tc.tile_pool(name="pp", bufs=2, space="PSUM") as pp,
            tc.tile_pool(name="op", bufs=4) as op,
        ):
            at_tiles = {}

            def load_at(k):
                t = atp.tile([P, SLOTS * P], mm_dt, tag=f"at{k}",
                             name=f"at{k}")
                nc.sync.dma_start(t[:], at_d[k * P:(k + 1) * P, :])
                at_tiles[k] = t

            for jt in range(NJT):
                psums = {}
                for k in range(4 * jt, KB):
                    if k not in at_tiles:
                        load_at(k)
                    bt = bp.tile([P, CW], mm_dt, tag="b", name=f"b{jt}_{k}")
                    nc.sync.dma_start(
                        bt[:], b_d[k * P:(k + 1) * P, jt * CW:(jt + 1) * CW])
                    for t in range(jt // 2, SLOTS):
                        kend = 8 * (t + 1)   # slot t accumulates k in [4*jt, kend)
                        if k >= kend:
                            continue
                        if k == 4 * jt:
                            psums[t] = pp.tile([P, CW], mybir.dt.float32,
                                               tag=f"ps{t}", name=f"ps{t}_{jt}")
                        nc.tensor.matmul(
                            psums[t][:],
                            lhsT=at_tiles[k][:, t * P:(t + 1) * P],
                            rhs=bt[:],
                            start=(k == 4 * jt),
                            stop=(k == kend - 1),
                        )
                        if k == kend - 1:
                            ot = op.tile([P, CW], mybir.dt.float32, tag="o",
                                         name=f"o{t}_{jt}")
                            nc.vector.tensor_copy(ot[:], psums[t][:])
                            nc.sync.dma_start(
                                o_d[t, :, jt * CW:(jt + 1) * CW], ot[:])

    nc.compile()
    return nc


def _get_nc(mm_dt_name):
    if mm_dt_name not in _cached:
        _cached[mm_dt_name] = _build(mm_dt_name)
    return _cached[mm_dt_name]


def kernel(A, B, mm_dt_name=MM_DT_NAME, trace=False):
    from concourse.bass_utils import run_bass_kernel_spmd

    A = np.ascontiguousarray(np.asarray(A, dtype=np.float32))
    B = np.ascontiguousarray(np.asarray(B, dtype=np.float32))
    AT = np.ascontiguousarray(A.T)  # AT[k, m] = A[m, k]

    nc = _get_nc(mm_dt_name)
    in_maps = []
    for c in range(NCORES):
        band = np.concatenate(
            [AT[:, (8 * t + c) * P:(8 * t + c + 1) * P] for t in range(SLOTS)],
            axis=1)
        in_maps.append({"at": np.ascontiguousarray(band), "b": B})

    res = run_bass_kernel_spmd(nc, in_maps, core_ids=list(range(NCORES)),
                               trace=trace)
    C = np.zeros((N, N), dtype=np.float32)
    for c in range(NCORES):
        o = res.results[c]["o"]
        for t in range(SLOTS):
            blk = 8 * t + c
            C[blk * P:(blk + 1) * P, :] = o[t]
    if trace:
        kernel.last_exec_time_ns = res.exec_time_ns
        kernel.last_results = res
    return C


# revision 8
# speedup vs baseline: 1.3578x; 1.3578x over previous
"""Trainium2 Bass kernel for C = tril(A @ B), A/B lower-triangular 4096x4096 fp32.

Distribution: 32 row-blocks of 128 interleaved mod 8 across cores; core c owns
blocks {c, 8+c, 16+c, 24+c} (slots t=0..3). Slot t uses a uniform K/column
bound of 8*(t+1) k-blocks so every core runs the identical SPMD program; the
inputs are exactly triangular, so the over-computed region contributes exact
zeros (no masking needed anywhere).

DMA layout: all inputs are repacked on the host partition-major so each DMA
moves >=0.5 MB with >=4 KB contiguous per partition:
  - A^T band: per-slot K-trimmed pack [128, sum_t 8(t+1)*128] (5 MB, 4 DMAs),
    SBUF-resident.
  - B: only the 144 nonzero-triangle tiles, packed per (col-tile jt) as
    4-k-block chunks [chunk, 128, 2048] (36 x 1MB DMAs, each partition row
    8 KB contiguous).
  - Output: PSUM -> SBUF staging [128, 1024] -> one DMA per 1024-col chunk.
"""

import numpy as np

N = 4096
P = 128
NCORES = 8
SLOTS = 4          # row-blocks per core
KB = N // P        # 32 k-blocks
CW = 512           # matmul free dim (fp32 max)
NJT = N // CW      # 8 col tiles
KC = 4             # k-blocks per B chunk

MM_DT_NAME = "float32r"  # "float32" (4 cyc/row) or "float32r" (1 cyc/row)

# slot t covers k in [0, 8*(t+1)); col tile jt active for slot t iff jt < 2*(t+1)
AT_KB = [8 * (t + 1) for t in range(SLOTS)]          # k-blocks per slot
AT_OFF = [sum(AT_KB[:t]) for t in range(SLOTS)]       # col offset in at_pack (k-blocks)
AT_TOT = sum(AT_KB)                                   # 80 k-blocks
B_CHUNKS = [(jt, 4 * jt + KC * cc) for jt in range(NJT)
            for cc in range((KB - 4 * jt) // KC)]     # (jt, kstart), 36 chunks

_cached = {}


def _build(mm_dt_name):
    import concourse.mybir as mybir
    import concourse.tile as tile
    from concourse import bacc

    mm_dt = getattr(mybir.dt, mm_dt_name)

    nc = bacc.Bacc("TRN2", target_bir_lowering=False, debug=False,
                   num_devices=NCORES)
    at_d = nc.dram_tensor("at", [P, AT_TOT * P], mm_dt,
                          kind="ExternalInput").ap()
    b_d = nc.dram_tensor("b", [len(B_CHUNKS) * P, KC * CW], mm_dt,
                         kind="ExternalInput").ap()
    o_d = nc.dram_tensor("o", [SLOTS, P, N], mybir.dt.float32,
                         kind="ExternalOutput").ap()

    with tile.TileContext(nc) as tc:
        with (
            tc.tile_pool(name="atp", bufs=1) as atp,
            tc.tile_pool(name="bp", bufs=4) as bp,
            tc.tile_pool(name="pp", bufs=2, space="PSUM") as pp,
            tc.tile_pool(name="sp", bufs=2) as sp,
        ):
            at_sb = []
            for t in range(SLOTS):
                a = atp.tile([P, AT_KB[t], P], mm_dt, tag=f"at{t}",
                             name=f"at{t}")
                nc.sync.dma_start(
                    a[:], at_d[:, AT_OFF[t] * P:(AT_OFF[t] + AT_KB[t]) * P])
                at_sb.append(a)

            psums = {}
            stages = {}
            for ci, (jt, kstart) in enumerate(B_CHUNKS):
                bch = bp.tile([P, KC, CW], mm_dt, tag="b", name=f"b{ci}")
                nc.sync.dma_start(bch[:], b_d[ci * P:(ci + 1) * P, :])
                for q in range(KC):
                    k = kstart + q
                    for t in range(jt // 2, SLOTS):
                        kend = 8 * (t + 1)
                        if k >= kend:
                            continue
                        if k == 4 * jt:
                            psums[t] = pp.tile([P, CW], mybir.dt.float32,
                                               tag=f"ps{t}", name=f"ps{t}_{jt}")
                        nc.tensor.matmul(
                            psums[t][:],
                            lhsT=at_sb[t][:, k, :],
                            rhs=bch[:, q, :],
                            start=(k == 4 * jt),
                            stop=(k == kend - 1),
                        )
                        if k == kend - 1:
                            u, half = jt // 2, jt % 2
                            if half == 0:
                                stages[t] = sp.tile([P, 2 * CW],
                                                    mybir.dt.float32,
                                                    tag=f"st{t}",
                                                    name=f"st{t}_{u}")
                            nc.vector.tensor_copy(
                                stages[t][:, half * CW:(half + 1) * CW],
                                psums[t][:])
                            if half == 1:
                                nc.sync.dma_start(
                                    o_d[t, :, u * 2 * CW:(u + 1) * 2 * CW],
                                    stages[t][:])

    nc.compile()
    return nc


def _get_nc(mm_dt_name):
    if mm_dt_name not in _cached:
        _cached[mm_dt_name] = _build(mm_dt_name)
    return _cached[mm_dt_name]


def _pack_b(B):
    """[36, 128, 2048]: chunk (jt, kstart) row p = 4 k-tiles' row p, concat."""
    B4 = B.reshape(KB, P, NJT, CW)
    slabs = [
        B4[ks:ks + KC, :, jt, :].transpose(1, 0, 2).reshape(P, KC * CW)
        for jt, ks in B_CHUNKS
    ]
    return np.ascontiguousarray(np.stack(slabs)).reshape(len(B_CHUNKS) * P,
                                                         KC * CW)


def _pack_at(A, c):
    """[128, 80*128]: slot t cols = A[block 8t+c rows, k<8(t+1)*128] as
    (p, k, m) with p the within-k-block row."""
    out = np.empty((P, AT_TOT * P), dtype=np.float32)
    for t in range(SLOTS):
        blk = 8 * t + c
        E = AT_KB[t] * P
        # block [m, kk] -> (p, k, m)
        blockT = A[blk * P:(blk + 1) * P, :E].T          # [kk, m]
        arr = blockT.reshape(AT_KB[t], P, P).transpose(1, 0, 2)  # [p, k, m]
        out[:, AT_OFF[t] * P:(AT_OFF[t] + AT_KB[t]) * P] = \
            arr.reshape(P, AT_KB[t] * P)
    return out


def kernel(A, B, mm_dt_name=MM_DT_NAME, trace=False):
    from concourse.bass_utils import run_bass_kernel_spmd

    A = np.ascontiguousarray(np.asarray(A, dtype=np.float32))
    B = np.ascontiguousarray(np.asarray(B, dtype=np.float32))

    nc = _get_nc(mm_dt_name)
    b_pack = _pack_b(B)
    in_maps = [{"at": _pack_at(A, c), "b": b_pack} for c in range(NCORES)]

    res = run_bass_kernel_spmd(nc, in_maps, core_ids=list(range(NCORES)),
                               trace=trace)
    C = np.zeros((N, N), dtype=np.float32)
    for c in range(NCORES):
        o = res.results[c]["o"]
        for t in range(SLOTS):
            blk = 8 * t + c
            C[blk * P:(blk + 1) * P, :] = o[t]
    if trace:
        kernel.last_exec_time_ns = res.exec_time_ns
        kernel.last_results = res
    return C


# revision 9
# speedup vs baseline: 1.6678x; 1.2283x over previous
"""Trainium2 Bass kernel for C = tril(A @ B), A/B lower-triangular 4096x4096 fp32.

Distribution (SPMD, 8 cores = 4 row-groups x 2 col-groups): core (g, h) owns
row-blocks {4t+g : t=0..7} (slots) and columns {512*(2l+h) : l=0..3} (locals).
Slot t uses a uniform K bound of 4*(t+1) k-blocks and local col l a uniform
K start of 8*l so every core runs the identical program; inputs are exactly
triangular, so all over-computed terms are exact zeros (no masking needed).

DMA layout: everything repacked on host, partition-major, so each DMA moves
>=0.5 MB with >=4 KB contiguous per partition:
  - A^T row-band: per-slot K-trimmed pack [128, 144*128] (9.4 MB, 8 DMAs),
    SBUF-resident.
  - B col-band: nonzero-triangle tiles packed per local col as 4-k-block
    chunks [20, 128, 2048] (20 x 1MB DMAs).
  - Output: PSUM -> SBUF staging pairs -> [128, 1024] DMA chunks.
"""

import numpy as np

N = 4096
P = 128
NCORES = 8
RG, CG = 4, 2       # row groups x col groups
SLOTS = N // P // RG    # 8 row-block slots per core
L = N // 512 // CG      # 4 local 512-col tiles per core
KB = N // P             # 32 k-blocks
CW = 512                # matmul free dim (fp32 max)
KC = 4                  # k-blocks per B chunk

MM_DT_NAME = "float32r"  # "float32" (4 cyc/row) or "float32r" (1 cyc/row)

AT_KB = [RG * (t + 1) for t in range(SLOTS)]          # k-blocks per slot
AT_OFF = [sum(AT_KB[:t]) for t in range(SLOTS)]
AT_TOT = sum(AT_KB)                                   # 144 k-blocks
B_CHUNKS = [(l, 8 * l + KC * cc) for l in range(L)
            for cc in range((KB - 8 * l) // KC)]      # (local col, kstart); 20


def _active(t, l):
    return 8 * l < RG * (t + 1)


_cached = {}


def _build(mm_dt_name):
    import concourse.mybir as mybir
    import concourse.tile as tile
    from concourse import bacc

    mm_dt = getattr(mybir.dt, mm_dt_name)

    nc = bacc.Bacc("TRN2", target_bir_lowering=False, debug=False,
                   num_devices=NCORES)
    at_d = nc.dram_tensor("at", [P, AT_TOT * P], mm_dt,
                          kind="ExternalInput").ap()
    b_d = nc.dram_tensor("b", [len(B_CHUNKS) * P, KC * CW], mm_dt,
                         kind="ExternalInput").ap()
    o_d = nc.dram_tensor("o", [SLOTS, P, L * CW], mybir.dt.float32,
                         kind="ExternalOutput").ap()

    with tile.TileContext(nc) as tc:
        with (
            tc.tile_pool(name="atp", bufs=1) as atp,
            tc.tile_pool(name="bp", bufs=5) as bp,
            tc.tile_pool(name="pp", bufs=1, space="PSUM") as pp,
            tc.tile_pool(name="sp", bufs=1) as sp,
        ):
            at_sb = []
            for t in range(SLOTS):
                a = atp.tile([P, AT_KB[t], P], mm_dt, tag=f"at{t}",
                             name=f"at{t}")
                nc.sync.dma_start(
                    a[:], at_d[:, AT_OFF[t] * P:(AT_OFF[t] + AT_KB[t]) * P])
                at_sb.append(a)

            psums = {}
            stages = {}
            for ci, (l, kstart) in enumerate(B_CHUNKS):
                bch = bp.tile([P, KC, CW], mm_dt, tag="b", name=f"b{ci}")
                nc.sync.dma_start(bch[:], b_d[ci * P:(ci + 1) * P, :])
                for q in range(KC):
                    k = kstart + q
                    for t in range(2 * l, SLOTS):
                        kend = RG * (t + 1)
                        if k >= kend:
                            continue
                        if k == 8 * l:
                            psums[t] = pp.tile([P, CW], mybir.dt.float32,
                                               tag=f"ps{t}", name=f"ps{t}_{l}")
                        nc.tensor.matmul(
                            psums[t][:],
                            lhsT=at_sb[t][:, k, :],
                            rhs=bch[:, q, :],
                            start=(k == 8 * l),
                            stop=(k == kend - 1),
                        )
                        if k == kend - 1:
                            u, half = l // 2, l % 2
                            if half == 0:
                                stages[t] = sp.tile([P, 2 * CW],
                                                    mybir.dt.float32,
                                                    tag=f"st{t}",
                                                    name=f"st{t}_{u}")
                            nc.vector.tensor_copy(
                                stages[t][:, half * CW:(half + 1) * CW],
                                psums[t][:])
                            last_of_pair = (half == 1) or not _active(t, l + 1)
                            if last_of_pair:
                                w = (half + 1) * CW
                                nc.sync.dma_start(
                                    o_d[t, :, u * 2 * CW:u * 2 * CW + w],
                                    stages[t][:, :w])

    nc.compile()
    return nc


def _get_nc(mm_dt_name):
    if mm_dt_name not in _cached:
        _cached[mm_dt_name] = _build(mm_dt_name)
    return _cached[mm_dt_name]


def _pack_b(B, h):
    """[20*128, 2048]: chunk (l, kstart) row p = 4 k-tiles' row p of global
    col-tile 2l+h, concatenated."""
    B4 = B.reshape(KB, P, N // CW, CW)
    slabs = [
        B4[ks:ks + KC, :, 2 * l + h, :].transpose(1, 0, 2).reshape(P, KC * CW)
        for l, ks in B_CHUNKS
    ]
    return np.ascontiguousarray(np.stack(slabs)).reshape(len(B_CHUNKS) * P,
                                                         KC * CW)


def _pack_at(A, g):
    """[128, 144*128]: slot t cols = A[block 4t+g rows, k < 4*(t+1)*128] laid
    out (p, k, m), p = row within k-block."""
    out = np.empty((P, AT_TOT * P), dtype=np.float32)
    for t in range(SLOTS):
        blk = RG * t + g
        E = AT_KB[t] * P
        blockT = A[blk * P:(blk + 1) * P, :E].T          # [kk, m]
        arr = blockT.reshape(AT_KB[t], P, P).transpose(1, 0, 2)
        out[:, AT_OFF[t] * P:(AT_OFF[t] + AT_KB[t]) * P] = \
            arr.reshape(P, AT_KB[t] * P)
    return out


def kernel(A, B, mm_dt_name=MM_DT_NAME, trace=False):
    from concourse.bass_utils import run_bass_kernel_spmd

    A = np.ascontiguousarray(np.asarray(A, dtype=np.float32))
    B = np.ascontiguousarray(np.asarray(B, dtype=np.float32))

    nc = _get_nc(mm_dt_name)
    b_packs = [_pack_b(B, h) for h in range(CG)]
    in_maps = [{"at": _pack_at(A, c % RG), "b": b_packs[c // RG]}
               for c in range(NCORES)]

    res = run_bass_kernel_spmd(nc, in_maps, core_ids=list(range(NCORES)),
                               trace=trace)
    C = np.zeros((N, N), dtype=np.float32)
    for c in range(NCORES):
        g, h = c % RG, c // RG
        o = res.results[c]["o"]
        for t in range(SLOTS):
            blk = RG * t + g
            for l in range(L):
                jt = 2 * l + h
                C[blk * P:(blk + 1) * P, jt * CW:(jt + 1) * CW] = \
                    o[t, :, l * CW:(l + 1) * CW]
    if trace:
        kernel.last_exec_time_ns = res.exec_time_ns
        kernel.last_results = res
    return C


# revision 12
# speedup vs baseline: 1.9292x; 1.1568x over previous
"""Trainium2 Bass kernel for C = tril(A @ B), A/B lower-triangular 4096x4096 fp32.

Distribution (SPMD, 8 cores = 4 row-groups x 2 col-groups): core (g, h) owns
row-blocks {4t+g : t=0..7} (slots) and columns {512*(2l+h) : l=0..3} (locals).
Slot t uses a uniform K bound of 4*(t+1) k-blocks and local col l a uniform
K start of 8*l so every core runs the identical program; inputs are exactly
triangular, so all over-computed terms are exact zeros (no masking needed).

DMA layout: everything repacked on host, partition-major, so each DMA moves
>=0.5 MB with >=4 KB contiguous per partition:
  - A^T row-band: per-slot K-trimmed pack [128, 144*128] (9.4 MB, 8 DMAs),
    SBUF-resident.
  - B col-band: nonzero-triangle tiles packed per local col as 4-k-block
    chunks [20, 128, 2048] (20 x 1MB DMAs).
  - Output: PSUM -> SBUF staging pairs -> [128, 1024] DMA chunks.
"""

import numpy as np

N = 4096
P = 128
NCORES = 8
RG, CG = 4, 2       # row groups x col groups
SLOTS = N // P // RG    # 8 row-block slots per core
L = N // 512 // CG      # 4 local 512-col tiles per core
KB = N // P             # 32 k-blocks
CW = 512                # matmul free dim (fp32 max)
KC = 4                  # k-blocks per B chunk

MM_DT_NAME = "float32r"  # "float32" (4 cyc/row) or "float32r" (1 cyc/row)

AT_KB = [RG * (t + 1) for t in range(SLOTS)]          # k-blocks per slot
AT_OFF = [sum(AT_KB[:t]) for t in range(SLOTS)]
AT_TOT = sum(AT_KB)                                   # 144 k-blocks
B_CHUNKS = [(l, 8 * l + KC * cc) for l in range(L)
            for cc in range((KB - 8 * l) // KC)]      # (local col, kstart); 20


def _active(t, l):
    return 8 * l < RG * (t + 1)


_cached = {}


def _build(mm_dt_name):
    import concourse.mybir as mybir
    import concourse.tile as tile
    from concourse import bacc

    mm_dt = getattr(mybir.dt, mm_dt_name)

    nc = bacc.Bacc("TRN2", target_bir_lowering=False, debug=False,
                   num_devices=NCORES)
    at_d = nc.dram_tensor("at", [P, AT_TOT * P], mm_dt,
                          kind="ExternalInput").ap()
    b_d = nc.dram_tensor("b", [len(B_CHUNKS) * P, KC * CW], mm_dt,
                         kind="ExternalInput").ap()
    o_d = nc.dram_tensor("o", [SLOTS, P, L * CW], mybir.dt.float32,
                         kind="ExternalOutput").ap()

    with tile.TileContext(nc) as tc:
        with (
            tc.tile_pool(name="atp", bufs=1) as atp,
            tc.tile_pool(name="bp", bufs=5) as bp,
            tc.tile_pool(name="pp", bufs=1, space="PSUM") as pp,
            tc.tile_pool(name="sp", bufs=1) as sp,
        ):
            at_sb = []
            for t in range(SLOTS):
                a = atp.tile([P, AT_KB[t], P], mm_dt, tag=f"at{t}",
                             name=f"at{t}")
                nc.sync.dma_start(
                    a[:], at_d[:, AT_OFF[t] * P:(AT_OFF[t] + AT_KB[t]) * P])
                at_sb.append(a)

            psums = {}
            stages = {}
            for ci, (l, kstart) in enumerate(B_CHUNKS):
                bch = bp.tile([P, KC, CW], mm_dt, tag="b", name=f"b{ci}")
                nc.sync.dma_start(bch[:], b_d[ci * P:(ci + 1) * P, :])
                for q in range(KC):
                    k = kstart + q
                    for t in range(2 * l, SLOTS):
                        kend = RG * (t + 1)
                        if k >= kend:
                            continue
                        if k == 8 * l:
                            psums[t] = pp.tile([P, CW], mybir.dt.float32,
                                               tag=f"ps{t}", name=f"ps{t}_{l}")
                        nc.tensor.matmul(
                            psums[t][:],
                            lhsT=at_sb[t][:, k, :],
                            rhs=bch[:, q, :],
                            start=(k == 8 * l),
                            stop=(k == kend - 1),
                        )
                        if k == kend - 1:
                            u, half = l // 2, l % 2
                            if half == 0:
                                stages[t] = sp.tile([P, 2 * CW],
                                                    mybir.dt.float32,
                                                    tag=f"st{t}",
                                                    name=f"st{t}_{u}")
                            nc.vector.tensor_copy(
                                stages[t][:, half * CW:(half + 1) * CW],
                                psums[t][:])
                            last_of_pair = (half == 1) or not _active(t, l + 1)
                            if last_of_pair:
                                w = (half + 1) * CW
                                nc.gpsimd.dma_start(
                                    o_d[t, :, u * 2 * CW:u * 2 * CW + w],
                                    stages[t][:, :w])

    nc.compile()
    return nc


def _get_nc(mm_dt_name):
    if mm_dt_name not in _cached:
        _cached[mm_dt_name] = _build(mm_dt_name)
    return _cached[mm_dt_name]


def _pack_b(B, h):
    """[20*128, 2048]: chunk (l, kstart) row p = 4 k-tiles' row p of global
    col-tile 2l+h, concatenated."""
    B4 = B.reshape(KB, P, N // CW, CW)
    slabs = [
        B4[ks:ks + KC, :, 2 * l + h, :].transpose(1, 0, 2).reshape(P, KC * CW)
        for l, ks in B_CHUNKS
    ]
    return np.ascontiguousarray(np.stack(slabs)).reshape(len(B_CHUNKS) * P,
                                                         KC * CW)


def _pack_at(A, g):
    """[128, 144*128]: slot t cols = A[block 4t+g rows, k < 4*(t+1)*128] laid
    out (p, k, m), p = row within k-block."""
    out = np.empty((P, AT_TOT * P), dtype=np.float32)
    for t in range(SLOTS):
        blk = RG * t + g
        E = AT_KB[t] * P
        blockT = A[blk * P:(blk + 1) * P, :E].T          # [kk, m]
        arr = blockT.reshape(AT_KB[t], P, P).transpose(1, 0, 2)
        out[:, AT_OFF[t] * P:(AT_OFF[t] + AT_KB[t]) * P] = \
            arr.reshape(P, AT_KB[t] * P)
    return out


def kernel(A, B, mm_dt_name=MM_DT_NAME, trace=False):
    from concourse.bass_utils import run_bass_kernel_spmd

    A = np.ascontiguousarray(np.asarray(A, dtype=np.float32))
    B = np.ascontiguousarray(np.asarray(B, dtype=np.float32))

    nc = _get_nc(mm_dt_name)
    b_packs = [_pack_b(B, h) for h in range(CG)]
    in_maps = [{"at": _pack_at(A, c % RG), "b": b_packs[c // RG]}
               for c in range(NCORES)]

    res = run_bass_kernel_spmd(nc, in_maps, core_ids=list(range(NCORES)),
                               trace=trace)
    C = np.zeros((N, N), dtype=np.float32)
    for c in range(NCORES):
        g, h = c % RG, c // RG
        o = res.results[c]["o"]
        for t in range(SLOTS):
            blk = RG * t + g
            for l in range(L):
                jt = 2 * l + h
                C[blk * P:(blk + 1) * P, jt * CW:(jt + 1) * CW] = \
                    o[t, :, l * CW:(l + 1) * CW]
    if trace:
        kernel.last_exec_time_ns = res.exec_time_ns
        kernel.last_results = res
    return C
